# revision 3
# baseline (speedup 1.0000x reference)
"""GCN graph-embedding kernel for 8 Trainium2 NeuronCores (Bass/Tile).

Strategy (dst-node sharding, per spec sharding_hint):
  - Nodes are permuted and bin-packed into 128-node blocks balanced by
    in-degree, 49 blocks per core (8 cores). Per-block edge lists (incl.
    self-loops) are padded to a uniform K tiles of 128 edges, so one SPMD
    program serves all cores.
  - Layer aggregation uses the identity  segment_sum(norm * x[src]) @ W =
    (sum_e norm_e x[src_e]) @ W : per 128-edge tile, source rows are fetched
    with an indirect DMA gather and combined on the TensorEngine with a
    selection matrix Sel[e, dstrel] = norm_e * (dstrel_e == col), accumulated
    in PSUM per destination block, then multiplied by the (replicated) weight
    matrix, biased and ReLU'd.
  - norm_e = rsqrt(deg[src] * deg[dst]) is computed on device from staged
    integer degree products (self-loops included, PyG GCNConv convention).
  - Between layers, each core's h-slice is exchanged with an AllGather so
    layer-2 gathers see the full node table (the halo is ~everything for a
    random graph, so a full exchange is optimal).
  - Global mean-pool is fused into layer 2 as a one-hot matmul accumulated in
    PSUM; partial per-core graph sums are combined with a small AllReduce and
    every core finishes the (tiny) linear head redundantly.

The walrus build in this container rejects instructions with more than one
semaphore wait; split_multi_waits() rewrites the scheduled program so each
instruction carries at most one (extra waits move to same-engine NoOps).
"""
import numpy as np

import concourse.bass as bass
import concourse.mybir as mybir
import concourse.tile as tile
from concourse.bass_utils import run_bass_kernel_spmd

F = 128          # feature width (all layers)
P = 128          # partitions / block size
CORES = 8
BPC = 49         # blocks per core
NG = 64          # number of graphs


def split_multi_waits(nc, max_waits: int = 1) -> int:
    n_split = 0
    f = nc.cur_f
    for bb in f.blocks:
        new_insts = []
        for inst in bb.instructions:
            si = inst.sync_info
            if si is not None and len(si.on_wait) > max_waits:
                waits = list(si.on_wait)
                extra, keep = waits[:-max_waits], waits[-max_waits:]
                for w in extra:
                    nop = mybir.InstNoOp(
                        name=nc.get_next_instruction_name(),
                        sync_info=mybir.SyncInfo(on_wait=[w], on_update=[]),
                        bass_nofuse=True,
                        engine=inst.engine,
                        ins=[],
                        outs=[],
                    )
                    nc.register_instruction(nop, overwrite=True)
                    new_insts.append(nop)
                inst.sync_info = mybir.SyncInfo(
                    on_wait=keep, on_update=list(si.on_update)
                )
                n_split += 1
            new_insts.append(inst)
        bb.instructions = new_insts
    return n_split


def _prep(x, edge_index, batch, n_cores=CORES, bpc=BPC, ng=NG):
    """Host-side integer/index preprocessing: node permutation + per-core
    edge tiling. No floating-point arithmetic on feature data."""
    import heapq

    n = x.shape[0]
    src = np.asarray(edge_index[0], dtype=np.int64)
    dst = np.asarray(edge_index[1], dtype=np.int64)
    deg = np.bincount(dst, minlength=n).astype(np.int64) + 1  # incl self-loop

    nblocks = n_cores * bpc
    cap = np.full(nblocks, P, dtype=np.int64)
    cap[bpc - 1::bpc] = P - 1  # reserve slot 127 of each core's last block
    assert cap.sum() >= n, "node capacity insufficient"

    # greedy balance: heaviest nodes first into the lightest non-full block
    order = np.argsort(-deg, kind="stable")
    heap = [(0, b) for b in range(nblocks)]
    heapq.heapify(heap)
    fill = np.zeros(nblocks, dtype=np.int64)
    node_block = np.empty(n, dtype=np.int64)
    node_slot = np.empty(n, dtype=np.int64)
    for nd in order:
        while True:
            load, b = heapq.heappop(heap)
            if fill[b] < cap[b]:
                break
        node_block[nd] = b
        node_slot[nd] = fill[b]
        fill[b] += 1
        if fill[b] < cap[b]:
            heapq.heappush(heap, (load + int(deg[nd]), b))

    vpad = nblocks * P
    pid = node_block * P + node_slot  # padded global id
    pad_pid = (bpc - 1) * P + (P - 1)  # core 0's reserved zero row

    # block edge loads (incl self-loops)
    eb = np.bincount(node_block[dst], minlength=nblocks)
    sb = np.bincount(node_block, minlength=nblocks)
    load_b = eb + sb
    K = int(np.ceil(load_b.max() / P))
    T = bpc * K

    # per-block edge arrays
    e_src_pid = pid[src]
    e_dst_block = node_block[dst]
    e_dst_slot = node_slot[dst]
    e_degp = deg[src] * deg[dst]

    # order edges by destination block for bucketed fill
    eorder = np.argsort(e_dst_block, kind="stable")
    es_pid = e_src_pid[eorder]
    es_slot = e_dst_slot[eorder]
    es_degp = e_degp[eorder]
    eb_cum = np.concatenate([[0], np.cumsum(eb[np.argsort(np.arange(nblocks))])])
    # eb_cum[b] = start of block b's edges in sorted order
    eb_cum = np.concatenate([[0], np.cumsum(eb)])

    # self edges grouped by block
    sorder = np.argsort(node_block, kind="stable")
    ss_pid = pid[sorder]
    ss_slot = node_slot[sorder]
    ss_degp = deg[sorder] * deg[sorder]
    sb_cum = np.concatenate([[0], np.cumsum(sb)])

    offs = np.full((n_cores, P, T), pad_pid, dtype=np.int32)
    dstrel = np.zeros((n_cores, P, T), dtype=np.float32)
    degp = np.ones((n_cores, P, T), dtype=np.float32)

    for b in range(nblocks):
        c, lb = divmod(b, bpc)
        p_s = np.concatenate([es_pid[eb_cum[b]:eb_cum[b + 1]],
                              ss_pid[sb_cum[b]:sb_cum[b + 1]]])
        p_r = np.concatenate([es_slot[eb_cum[b]:eb_cum[b + 1]],
                              ss_slot[sb_cum[b]:sb_cum[b + 1]]])
        p_d = np.concatenate([es_degp[eb_cum[b]:eb_cum[b + 1]],
                              ss_degp[sb_cum[b]:sb_cum[b + 1]]])
        m = len(p_s)
        assert m <= K * P
        rows = np.arange(m) % P
        cols = lb * K + np.arange(m) // P
        offs[c, rows, cols] = p_s
        dstrel[c, rows, cols] = p_r
        degp[c, rows, cols] = p_d

    # padded feature table
    x_pad = np.zeros((vpad, F), dtype=np.float32)
    x_pad[pid] = np.asarray(x, dtype=np.float32)

    # per-core batch table [P, bpc] (graph id per slot, -1 for pads)
    batchp = np.full((n_cores, P, bpc), -1.0, dtype=np.float32)
    bt = np.asarray(batch, dtype=np.int64)
    for c in range(n_cores):
        mask = (node_block >= c * bpc) & (node_block < (c + 1) * bpc)
        nb = node_block[mask] - c * bpc
        sl = node_slot[mask]
        batchp[c, sl, nb] = bt[mask].astype(np.float32)

    cnt = np.bincount(bt, minlength=ng).astype(np.float32)[:, None]  # [ng,1]
    return dict(offs=offs, dstrel=dstrel, degp=degp, batchp=batchp, cnt=cnt,
                x_pad=x_pad, K=K, T=T, vpad=vpad)


def _build(K, T, vpad, n_cores=CORES, bpc=BPC, ng=NG):
    f32 = mybir.dt.float32
    AF = mybir.ActivationFunctionType
    nc = bass.Bass()

    xp = nc.declare_dram_parameter("x_pad", [vpad, F], f32, isOutput=False)
    offs_p = nc.declare_dram_parameter("offs", [P, T], mybir.dt.int32,
                                       isOutput=False)
    dstrel_p = nc.declare_dram_parameter("dstrel", [P, T], f32, isOutput=False)
    degp_p = nc.declare_dram_parameter("degp", [P, T], f32, isOutput=False)
    batch_p = nc.declare_dram_parameter("batchp", [P, bpc], f32, isOutput=False)
    cnt_p = nc.declare_dram_parameter("cnt", [ng, 1], f32, isOutput=False)
    iota_p = nc.declare_dram_parameter("iota", [P, P], f32, isOutput=False)
    w1_p = nc.declare_dram_parameter("W1", [F, F], f32, isOutput=False)
    w2_p = nc.declare_dram_parameter("W2", [F, F], f32, isOutput=False)
    wl_p = nc.declare_dram_parameter("Wl", [F, F], f32, isOutput=False)
    b1_p = nc.declare_dram_parameter("b1bc", [P, F], f32, isOutput=False)
    b2_p = nc.declare_dram_parameter("b2bc", [P, F], f32, isOutput=False)
    bl_p = nc.declare_dram_parameter("blbc", [ng, F], f32, isOutput=False)
    out_p = nc.declare_dram_parameter("out", [ng, F], f32, isOutput=True)

    slice_rows = bpc * P

    with tile.TileContext(nc) as tc:
        with (
            tc.tile_pool(name="dram", bufs=1, space="DRAM") as dram,
            tc.tile_pool(name="const", bufs=1) as cp,
            tc.tile_pool(name="gp", bufs=6) as gp,
            tc.tile_pool(name="ip", bufs=6) as ipool,
            tc.tile_pool(name="sp", bufs=6) as spool,
            tc.tile_pool(name="bp", bufs=3) as bpool,
            tc.tile_pool(name="ps", bufs=2, space="PSUM") as psp,
            tc.tile_pool(name="psacc", bufs=1, space="PSUM") as psacc,
        ):
            ag_in = dram.tile([slice_rows, F], f32)
            h_tab = dram.tile([vpad, F], f32)
            ar_in = dram.tile([F, ng], f32)
            ar_out = dram.tile([F, ng], f32)

            # bulk constant loads
            offs_sb = cp.tile([P, T], mybir.dt.int32)
            nc.sync.dma_start(out=offs_sb[:], in_=offs_p[:])
            dstrel_sb = cp.tile([P, T], f32)
            nc.sync.dma_start(out=dstrel_sb[:], in_=dstrel_p[:])
            degp_sb = cp.tile([P, T], f32)
            nc.sync.dma_start(out=degp_sb[:], in_=degp_p[:])
            rdeg = cp.tile([P, T], f32)
            nc.vector.reciprocal(out=rdeg[:], in_=degp_sb[:])
            normc = cp.tile([P, T], f32)
            nc.scalar.activation(out=normc[:], in_=rdeg[:], func=AF.Sqrt)
            batch_sb = cp.tile([P, bpc], f32)
            nc.sync.dma_start(out=batch_sb[:], in_=batch_p[:])
            iota_sb = cp.tile([P, P], f32)
            nc.sync.dma_start(out=iota_sb[:], in_=iota_p[:])
            w1_sb = cp.tile([F, F], f32)
            nc.sync.dma_start(out=w1_sb[:], in_=w1_p[:])
            w2_sb = cp.tile([F, F], f32)
            nc.sync.dma_start(out=w2_sb[:], in_=w2_p[:])
            wl_sb = cp.tile([F, F], f32)
            nc.sync.dma_start(out=wl_sb[:], in_=wl_p[:])
            b1_sb = cp.tile([P, F], f32)
            nc.sync.dma_start(out=b1_sb[:], in_=b1_p[:])
            b2_sb = cp.tile([P, F], f32)
            nc.sync.dma_start(out=b2_sb[:], in_=b2_p[:])
            bl_sb = cp.tile([ng, F], f32)
            nc.sync.dma_start(out=bl_sb[:], in_=bl_p[:])
            cnt_sb = cp.tile([ng, 1], f32)
            nc.sync.dma_start(out=cnt_sb[:], in_=cnt_p[:])
            zsb = cp.tile([1, F], f32)
            nc.vector.memset(zsb[:], 0.0)

            pool_acc = psacc.tile([F, ng], f32)

            def layer(src_tab, w_sb, bbc_sb, is_last):
                for b in range(bpc):
                    psum_agg = psp.tile([F, P], f32, tag="agg")
                    for k in range(K):
                        t = b * K + k
                        g = gp.tile([P, F], f32, tag="g")
                        nc.gpsimd.indirect_dma_start(
                            out=g[:],
                            out_offset=None,
                            in_=src_tab,
                            in_offset=bass.IndirectOffsetOnAxis(
                                ap=offs_sb[:, t:t + 1], axis=0),
                        )
                        iseq = ipool.tile([P, P], f32, tag="iseq")
                        nc.vector.tensor_tensor(
                            out=iseq[:],
                            in0=dstrel_sb[:, t:t + 1].to_broadcast([P, P]),
                            in1=iota_sb[:],
                            op=mybir.AluOpType.is_equal,
                        )
                        sel = spool.tile([P, P], f32, tag="sel")
                        nc.scalar.activation(
                            out=sel[:], in_=iseq[:], func=AF.Copy,
                            scale=normc[:, t:t + 1],
                        )
                        # aggT[fi, d] += g.T @ sel
                        nc.tensor.matmul(
                            out=psum_agg[:], lhsT=g[:], rhs=sel[:],
                            start=(k == 0), stop=(k == K - 1),
                        )
                    aggT_sb = bpool.tile([F, P], f32, tag="aggT")
                    nc.vector.tensor_copy(out=aggT_sb[:], in_=psum_agg[:])
                    psum_h = psp.tile([P, F], f32, tag="h")
                    nc.tensor.matmul(out=psum_h[:], lhsT=aggT_sb[:], rhs=w_sb[:],
                                     start=True, stop=True)
                    hb = bpool.tile([P, F], f32, tag="hb")
                    nc.vector.tensor_add(out=hb[:], in0=psum_h[:], in1=bbc_sb[:])
                    hr = bpool.tile([P, F], f32, tag="hr")
                    nc.scalar.activation(out=hr[:], in_=hb[:], func=AF.Relu)
                    if not is_last:
                        nc.sync.dma_start(
                            out=ag_in[b * P:(b + 1) * P, :], in_=hr[:])
                    else:
                        gb = bpool.tile([P, ng], f32, tag="G")
                        nc.vector.tensor_tensor(
                            out=gb[:],
                            in0=batch_sb[:, b:b + 1].to_broadcast([P, ng]),
                            in1=iota_sb[:, :ng],
                            op=mybir.AluOpType.is_equal,
                        )
                        # poolT[fo, g] += hr.T @ gb
                        nc.tensor.matmul(out=pool_acc[:], lhsT=hr[:], rhs=gb[:],
                                         start=(b == 0), stop=(b == bpc - 1))

            # ---- layer 1 ----
            layer(xp[:], w1_sb, b1_sb, is_last=False)
            # guarantee the reserved pad row is zero in the exchanged table
            nc.sync.dma_start(
                out=ag_in[(bpc - 1) * P + P - 1:(bpc - 1) * P + P, :],
                in_=zsb[0:1, :])
            nc.gpsimd.collective_compute(
                "AllGather",
                mybir.AluOpType.bypass,
                replica_groups=[list(range(n_cores))],
                ins=[ag_in.opt()],
                outs=[h_tab.opt()],
            )
            # ---- layer 2 + fused mean-pool partials ----
            layer(h_tab[:], w2_sb, b2_sb, is_last=True)

            poolT_sb = cp.tile([F, ng], f32)
            nc.vector.tensor_copy(out=poolT_sb[:], in_=pool_acc[:])
            nc.gpsimd.dma_start(out=ar_in[:], in_=poolT_sb[:])
            nc.gpsimd.collective_compute(
                "AllReduce",
                mybir.AluOpType.add,
                replica_groups=[list(range(n_cores))],
                ins=[ar_in.opt()],
                outs=[ar_out.opt()],
            )
            poolT_ar = cp.tile([F, ng], f32)
            nc.gpsimd.dma_start(out=poolT_ar[:], in_=ar_out[:])

            # head: out[g, :] = (sums[g] / max(cnt,1)) @ Wl + bl
            psum_o = psp.tile([ng, F], f32, tag="o")
            nc.tensor.matmul(out=psum_o[:], lhsT=poolT_ar[:], rhs=wl_sb[:],
                             start=True, stop=True)
            cmax = cp.tile([ng, 1], f32)
            nc.vector.tensor_scalar(out=cmax[:], in0=cnt_sb[:], scalar1=1.0,
                                    scalar2=None, op0=mybir.AluOpType.max)
            rcnt = cp.tile([ng, 1], f32)
            nc.vector.reciprocal(out=rcnt[:], in_=cmax[:])
            osc = cp.tile([ng, F], f32)
            nc.scalar.activation(out=osc[:], in_=psum_o[:], func=AF.Copy,
                                 scale=rcnt[:])
            ofin = cp.tile([ng, F], f32)
            nc.vector.tensor_add(out=ofin[:], in0=osc[:], in1=bl_sb[:])
            nc.sync.dma_start(out=out_p[:], in_=ofin[:])

    split_multi_waits(nc)
    return nc


def _run(inputs, trace=False, n_cores=CORES, bpc=BPC):
    x = np.asarray(inputs["x"], dtype=np.float32)
    edge_index = np.asarray(inputs["edge_index"])
    batch = np.asarray(inputs["batch"])
    ng = NG
    pp = _prep(x, edge_index, batch, n_cores=n_cores, bpc=bpc, ng=ng)

    iota = np.tile(np.arange(P, dtype=np.float32), (P, 1))
    w1 = np.asarray(inputs["W1"], dtype=np.float32)
    w2 = np.asarray(inputs["W2"], dtype=np.float32)
    wl = np.asarray(inputs["Wl"], dtype=np.float32)
    b1bc = np.tile(np.asarray(inputs["b1"], dtype=np.float32), (P, 1))
    b2bc = np.tile(np.asarray(inputs["b2"], dtype=np.float32), (P, 1))
    blbc = np.tile(np.asarray(inputs["bl"], dtype=np.float32), (ng, 1))

    nc = _build(pp["K"], pp["T"], pp["vpad"], n_cores=n_cores, bpc=bpc, ng=ng)
    in_maps = []
    for c in range(n_cores):
        in_maps.append({
            "x_pad": pp["x_pad"],
            "offs": pp["offs"][c],
            "dstrel": pp["dstrel"][c],
            "degp": pp["degp"][c],
            "batchp": pp["batchp"][c],
            "cnt": pp["cnt"],
            "iota": iota,
            "W1": w1, "W2": w2, "Wl": wl,
            "b1bc": b1bc, "b2bc": b2bc, "blbc": blbc,
        })
    res = run_bass_kernel_spmd(nc, in_maps, list(range(n_cores)), trace=trace)
    return res.results[0]["out"], res.exec_time_ns


def kernel(**inputs) -> np.ndarray:
    out, _ = _run(inputs)
    return out
